# revision 2
# baseline (speedup 1.0000x reference)
"""Trainium2 Bass kernel for MoE soft-routed classification head.

Reference math (B=32, S=128, H=1024, E=16, L=8):
    sel_dw = einsum('be,eoh->boh', gates, dense_w)
    sel_db = einsum('be,eh->bh',  gates, dense_b)
    sel_ow = einsum('be,elh->blh', gates, out_proj_w)
    sel_ob = einsum('be,el->bl',  gates, out_proj_b)
    x   = X[:, 0, :]
    h   = tanh(einsum('bh,boh->bo', x, sel_dw) + sel_db)
    out = einsum('bh,blh->bl', h, sel_ow) + sel_ob

Key reordering (same as the fp16 baseline):
  h_pre[b,o] = sum_{e,h} (gates[b,e]*x[b,h]) * dense_w[e,o,h]
             + sum_e gates[b,e]*dense_b[e,o]
so with Z[(e,h),b] = gates[b,e]*x[b,h] stage 1 is ONE matmul with
contraction K = E*H (+1 bias chunk), and only the CLS token of X is
touched.  dense_w's output dim o (H=1024) is split 128-per-core across
8 cores; each core computes h[:, o_slice] end-to-end plus its partial
of the final [B,L]; the host sums partials and adds gates@out_proj_b.

What's new vs the fp16 baseline (26.7us):
  * dense_w streams as INT8 with a per-expert symmetric scale s_e =
    absmax_e/127.  The scale folds into the gates used for Z (free),
    so the PE still computes the exact reordered sum, just with
    quantized weights.  Measured end-to-end rel-err 1.42e-2 (< 2e-2
    gate), and HBM traffic halves: 2.1 MB/core instead of 4.2 MB.
  * The int8->fp16 conversion rides the DMA itself: gpsimd (SWDGE)
    DMAs may cast dtypes in flight, so the PE consumes plain fp16
    tiles and HBM only ever sees 1 byte/element.
  * Z rows are pre-scaled by C=512 (tanh applies scale=1/C) so the
    tiny g*s_e*x products stay in fp16 normal range.
  * Expert 0's Z block is computed on the host and lands with the
    first small DMA, so the first matmuls need no DVE dependency;
    DVE builds experts 1-15 while the weight stream runs.
  * The tanh activation table is preloaded at t=0 via a dummy
    activation so the real tanh doesn't pay the 1.3us table load.
"""

import contextlib
import ctypes
import os
import sys
import types

import numpy as np


def _install_ntff_shim():
    """Provide antenv.axon_hooks if the image's antenv lacks it."""
    try:
        import antenv.axon_hooks  # noqa: F401
        return
    except ImportError:
        pass

    so_path = "/opt/axon/libaxon_pjrt.so"
    hook = None
    if os.path.exists(so_path):
        try:
            lib = ctypes.CDLL(so_path)
            if hasattr(lib, "axon_start_nrt_profile"):
                lib.axon_start_nrt_profile.argtypes = [
                    ctypes.POINTER(ctypes.c_int64), ctypes.c_size_t]
                lib.axon_start_nrt_profile.restype = ctypes.c_int64
                lib.axon_stop_nrt_profile.argtypes = [ctypes.c_char_p]
                lib.axon_stop_nrt_profile.restype = ctypes.c_int64

                @contextlib.contextmanager
                def _hook(output_dir, device_ids):
                    import jax
                    jax.devices()
                    if device_ids:
                        ids = (ctypes.c_int64 * len(device_ids))(*device_ids)
                        rc = lib.axon_start_nrt_profile(ids, len(device_ids))
                    else:
                        rc = lib.axon_start_nrt_profile(None, 0)
                    if rc != 0:
                        raise RuntimeError(f"axon_start_nrt_profile rc={rc}")
                    try:
                        yield
                    finally:
                        n = lib.axon_stop_nrt_profile(str(output_dir).encode())
                        print(f"ntff profile: {n} file(s) -> {output_dir}",
                              file=sys.stderr)

                hook = _hook
        except OSError:
            pass

    mod = types.ModuleType("antenv.axon_hooks")
    mod._hook = hook
    mod.set_axon_ntff_profile_hook = lambda h: setattr(mod, "_hook", h)
    mod.get_axon_ntff_profile_hook = lambda: mod._hook
    sys.modules["antenv.axon_hooks"] = mod


_install_ntff_shim()

B, S, H, E, L = 32, 128, 1024, 16, 8
NCORES = 8
OSL = H // NCORES            # 128 output columns of dense layer per core
NHC = H // 128               # 8 h-chunks per expert
NWC = E * NHC                # 128 int8 weight chunks
EL = E * L                   # 128
CSC = 512.0                  # Z pre-scale; tanh applies 1/CSC

# xg fp16 packed layout [128, XGW] column offsets
XG_XT = 0                    # xt[p, hc*B+b] = x[b, hc*128+p]        (NHC*B)
XG_GS = XG_XT + NHC * B      # gs[p, e*B+b] = gates[b,e]*s_e*CSC     (E*B)
XG_Z0 = XG_GS + E * B        # z0[p, hc*B+b] = xt*gs[e=0]  (host)    (NHC*B)
XG_ZT = XG_Z0 + NHC * B      # ztail[p, b] = gates[b,p]*CSC, p<E     (B)
XG_WB = XG_ZT + B            # wb[p, j] = dense_b[p, osl[j]], p<E    (OSL)
XG_OW = XG_WB + OSL          # ow[p, l*E+e] = out_proj_w[e,l,osl[p]] (EL)
XG_GX = XG_OW + EL           # gex[p, l*E+e] = gates[p,e], p<B       (EL)
XGW = XG_GX + EL             # 1440

# int8 weight chunk groups on the gpsimd casting-DMA queue
W_GROUPS = [8, 24, 40, 56]
assert sum(W_GROUPS) == NWC

_CACHE = {}
LAST_RESULTS = None


def _build_nc():
    import concourse.bacc as bacc
    import concourse.tile as tile
    import concourse.mybir as mybir

    f16 = mybir.dt.float16
    f32 = mybir.dt.float32
    i8 = mybir.dt.int8

    nc = bacc.Bacc("TRN2", target_bir_lowering=False, debug=False,
                   num_devices=NCORES)

    w8_d = nc.dram_tensor("w8", [128, NWC * OSL], i8, kind="ExternalInput")
    xg_d = nc.dram_tensor("xg", [128, XGW], f16, kind="ExternalInput")
    out_d = nc.dram_tensor("out", [B, L], f32, kind="ExternalOutput")

    with tile.TileContext(nc) as tc:
        with (
            tc.tile_pool(name="const", bufs=1) as cpool,
            tc.tile_pool(name="wz", bufs=1) as wpool,
            tc.tile_pool(name="work", bufs=1) as spool,
            tc.tile_pool(name="psum", bufs=1, space="PSUM") as ppool,
        ):
            # Preload the tanh activation table at t=0 (dummy activation on
            # a memset tile) so the real tanh at the tail skips the ~1.3us
            # ACT_TABLE_LOAD.
            dz = spool.tile([1, 1], f32)
            nc.vector.memset(dz[:], 0.0)
            dzo = spool.tile([1, 1], f16)
            nc.scalar.activation(dzo[:], dz[:],
                                 mybir.ActivationFunctionType.Tanh)

            # Small fp16 tensors on the two HWDGE queues.
            z0_sb = cpool.tile([128, NHC * B], f16)
            nc.sync.dma_start(z0_sb[:], xg_d[:, XG_Z0:XG_Z0 + NHC * B])
            xtgs_sb = cpool.tile([128, XG_Z0], f16)
            nc.scalar.dma_start(xtgs_sb[:], xg_d[:, :XG_Z0])
            tail_sb = cpool.tile([128, XGW - XG_ZT], f16)
            nc.sync.dma_start(tail_sb[:], xg_d[:, XG_ZT:])

            # int8 weight chunks stream on gpsimd casting DMAs -> fp16 SBUF.
            wts = []
            c0 = 0
            for g, n_c in enumerate(W_GROUPS):
                wt = wpool.tile([128, n_c * OSL], f16, tag=f"wt{g}",
                                name=f"wt{g}")
                nc.gpsimd.dma_start(wt[:], w8_d[:, c0 * OSL:(c0 + n_c) * OSL])
                wts.append((c0, n_c, wt))
                c0 += n_c

            # Z for experts 1-15 on DVE (expert 0 came from the host).
            zt_sb = spool.tile([128, (E - 1) * NHC * B], f16)
            xt3 = xtgs_sb[:, XG_XT:XG_XT + NHC * B].rearrange(
                "p (h b) -> p h b", b=B)
            for e in range(1, E):
                g_b = (
                    xtgs_sb[:, XG_GS + e * B:XG_GS + (e + 1) * B]
                    .unsqueeze(1)
                    .to_broadcast((128, NHC, B))
                )
                nc.vector.tensor_mul(
                    zt_sb[:, (e - 1) * NHC * B:e * NHC * B].rearrange(
                        "p (h b) -> p h b", b=B),
                    xt3,
                    g_b,
                )

            # Stage 1: h_preT[o, b] over 128 int8 chunks + 1 fp16 bias chunk.
            ps1 = ppool.tile([OSL, B], f32)
            for c0, n_c, wt in wts:
                for i in range(n_c):
                    c = c0 + i
                    if c < NHC:
                        rhs = z0_sb[:, c * B:(c + 1) * B]
                    else:
                        rhs = zt_sb[:, (c - NHC) * B:(c - NHC + 1) * B]
                    nc.tensor.matmul(
                        ps1[:],
                        wt[:, i * OSL:(i + 1) * OSL],
                        rhs,
                        start=(c == 0),
                        stop=False,
                    )
            nc.tensor.matmul(
                ps1[:],
                tail_sb[:, XG_WB - XG_ZT:XG_WB - XG_ZT + OSL],
                tail_sb[:, 0:B],
                start=False,
                stop=True,
            )

            ht = spool.tile([OSL, B], f16)
            nc.scalar.activation(ht[:], ps1[:],
                                 mybir.ActivationFunctionType.Tanh,
                                 scale=1.0 / CSC)

            ps2 = ppool.tile([B, EL], f32)
            nc.tensor.matmul(
                ps2[:], ht[:],
                tail_sb[:, XG_OW - XG_ZT:XG_OW - XG_ZT + EL],
                start=True, stop=True)

            # r[b,(l,e)] = ps2 * gates[b,e]; reduce over e (innermost).
            r = spool.tile([B, EL], f32)
            nc.vector.tensor_mul(
                r[:], ps2[:], tail_sb[0:B, XG_GX - XG_ZT:XG_GX - XG_ZT + EL])
            out_r = spool.tile([B, L], f32)
            nc.vector.tensor_reduce(
                out_r[:],
                r[:].rearrange("p (l e) -> p l e", e=E),
                axis=mybir.AxisListType.X,
                op=mybir.AluOpType.add,
            )
            nc.sync.dma_start(out_d[:], out_r[:])

    nc.compile()
    return nc


def _get_nc():
    if "nc" not in _CACHE:
        _CACHE["nc"] = _build_nc()
    return _CACHE["nc"]


def make_in_maps(X, gates, dense_w, dense_b, out_proj_w, out_proj_b):
    """Host-side shard + pack. Returns (in_maps, host_bias)."""
    X = np.asarray(X, np.float32)
    gates = np.asarray(gates, np.float32)
    dense_w = np.asarray(dense_w, np.float32)
    dense_b = np.asarray(dense_b, np.float32)
    out_proj_w = np.asarray(out_proj_w, np.float32)
    out_proj_b = np.asarray(out_proj_b, np.float32)

    x = X[:, 0, :]                                     # [B, H]

    # Per-expert int8 symmetric quantization; scale folds into the gates
    # used for Z generation.
    s_e = np.abs(dense_w).max(axis=(1, 2)) / 127.0     # [E]
    w_q = np.clip(np.rint(dense_w / s_e[:, None, None]), -127, 127)
    dw_t = w_q.transpose(0, 2, 1)                      # [E, h, o] int values

    gs = gates * s_e[None, :] * CSC                    # [B, E] scaled gates

    # xt[p, hc*B+b] = x[b, hc*128+p]
    xt = x.T.reshape(NHC, 128, B).transpose(1, 0, 2).reshape(128, NHC * B)

    xg = np.zeros((128, XGW), np.float16)
    xg[:, XG_XT:XG_XT + NHC * B] = xt
    xg[:, XG_GS:XG_GS + E * B] = np.broadcast_to(
        gs.T.reshape(1, E * B), (128, E * B))
    # Host Z for expert 0 — from the fp16-rounded xt/gs so it matches what
    # the device would compute.
    xt16 = xg[:, XG_XT:XG_XT + NHC * B].astype(np.float32)
    gs0 = xg[:, XG_GS:XG_GS + B].astype(np.float32)    # e=0 block, any row
    z0 = (xt16.reshape(128, NHC, B) * gs0[:, None, :]).reshape(128, NHC * B)
    xg[:, XG_Z0:XG_Z0 + NHC * B] = z0
    xg[:E, XG_ZT:XG_ZT + B] = gates.T * CSC            # bias-z rows
    # l-major expert expansion: gex[b, l*E+e] = gates[b, e]
    xg[:B, XG_GX:XG_GX + EL] = np.tile(gates, (1, L))

    in_maps = []
    for k in range(NCORES):
        sl = slice(k * OSL, (k + 1) * OSL)
        # w8[p, c*OSL + j] = chunk c's rows: chunk (e, hc) at c = e*NHC+hc,
        # value dw_t[e, hc*128+p, sl][j]
        w8 = np.ascontiguousarray(
            dw_t[:, :, sl]                              # [E, H, OSL]
            .reshape(E, NHC, 128, OSL)
            .transpose(2, 0, 1, 3)                      # [128, E, NHC, OSL]
            .reshape(128, NWC * OSL)
        ).astype(np.int8)

        xgk = xg.copy()
        xgk[:E, XG_WB:XG_WB + OSL] = dense_b[:, sl]
        # ow[p, l*E+e] = out_proj_w[e, l, sl[p]]
        xgk[:, XG_OW:XG_OW + EL] = (
            out_proj_w[:, :, sl].transpose(2, 1, 0).reshape(OSL, EL)
        )

        in_maps.append({"w8": w8, "xg": xgk})

    host_bias = (gates @ out_proj_b).astype(np.float32)   # [B, L]
    return in_maps, host_bias


def kernel(**inputs):
    global LAST_RESULTS
    from concourse.bass_utils import run_bass_kernel_spmd

    nc = _get_nc()
    in_maps, host_bias = make_in_maps(
        inputs["X"], inputs["gates"], inputs["dense_w"], inputs["dense_b"],
        inputs["out_proj_w"], inputs["out_proj_b"],
    )
    res = run_bass_kernel_spmd(nc, in_maps, list(range(NCORES)))
    LAST_RESULTS = res
    parts = [r["out"] for r in res.results]
    out = np.sum(parts, axis=0, dtype=np.float64).astype(np.float32) + host_bias
    return out


# revision 4
# speedup vs baseline: 1.0109x; 1.0109x over previous
"""Trainium2 Bass kernel for MoE soft-routed classification head.

Reference math (B=32, S=128, H=1024, E=16, L=8):
    sel_dw = einsum('be,eoh->boh', gates, dense_w)
    sel_db = einsum('be,eh->bh',  gates, dense_b)
    sel_ow = einsum('be,elh->blh', gates, out_proj_w)
    sel_ob = einsum('be,el->bl',  gates, out_proj_b)
    x   = X[:, 0, :]
    h   = tanh(einsum('bh,boh->bo', x, sel_dw) + sel_db)
    out = einsum('bh,blh->bl', h, sel_ow) + sel_ob

Key reordering:
  h_pre[b,o] = sum_{e,h} (gates[b,e]*x[b,h]) * dense_w[e,o,h]
             + sum_e gates[b,e]*dense_b[e,o]
so with Z[(e,h),b] = gates[b,e]*x[b,h] stage 1 is ONE matmul with
contraction K = E*H (+1 bias chunk); only the CLS token of X is
touched.  dense_w's output dim o (H=1024) is split 128-per-core across
8 cores; each core computes h[:, o_slice] end-to-end plus its partial
of the final [B,L]; the host sums partials and adds gates@out_proj_b.

Schedule (from trace analysis of the 26.7us baseline):
  * The weight stream is DMA-DEST-byte bound at ~360 GB/s/core, so all
    fp16 weight bytes take ~11.6us no matter how they're queued.  The
    wins are at the edges: issue the stream as the FIRST instructions
    after engine boot on ONE HWDGE queue (deterministic FIFO landing),
    keep the DMA-instruction count low (every HW DMA burns a DMAHW
    semaphore lane and the kernel-exit sem sync pays per sem), and
    make the LAST weight group small so the PE finishes right behind
    the stream.
  * Z is generated on DVE from a small packed tensor (x chunks +
    pre-scaled gate broadcasts) that lands before the first big weight
    group; 16 per-expert muls finish long before their chunks arrive.
  * Weights are pre-scaled by CSC=512 via the gates so the g*x products
    stay in fp16 normal range; tanh applies scale=1/CSC.
  * The tanh table is preloaded at t=0 via a dummy activation.
"""

import contextlib
import ctypes
import os
import sys
import types

import numpy as np


def _install_ntff_shim():
    """Provide antenv.axon_hooks if the image's antenv lacks it."""
    try:
        import antenv.axon_hooks  # noqa: F401
        return
    except ImportError:
        pass

    so_path = "/opt/axon/libaxon_pjrt.so"
    hook = None
    if os.path.exists(so_path):
        try:
            lib = ctypes.CDLL(so_path)
            if hasattr(lib, "axon_start_nrt_profile"):
                lib.axon_start_nrt_profile.argtypes = [
                    ctypes.POINTER(ctypes.c_int64), ctypes.c_size_t]
                lib.axon_start_nrt_profile.restype = ctypes.c_int64
                lib.axon_stop_nrt_profile.argtypes = [ctypes.c_char_p]
                lib.axon_stop_nrt_profile.restype = ctypes.c_int64

                @contextlib.contextmanager
                def _hook(output_dir, device_ids):
                    import jax
                    jax.devices()
                    if device_ids:
                        ids = (ctypes.c_int64 * len(device_ids))(*device_ids)
                        rc = lib.axon_start_nrt_profile(ids, len(device_ids))
                    else:
                        rc = lib.axon_start_nrt_profile(None, 0)
                    if rc != 0:
                        raise RuntimeError(f"axon_start_nrt_profile rc={rc}")
                    try:
                        yield
                    finally:
                        n = lib.axon_stop_nrt_profile(str(output_dir).encode())
                        print(f"ntff profile: {n} file(s) -> {output_dir}",
                              file=sys.stderr)

                hook = _hook
        except OSError:
            pass

    mod = types.ModuleType("antenv.axon_hooks")
    mod._hook = hook
    mod.set_axon_ntff_profile_hook = lambda h: setattr(mod, "_hook", h)
    mod.get_axon_ntff_profile_hook = lambda: mod._hook
    sys.modules["antenv.axon_hooks"] = mod


_install_ntff_shim()

B, S, H, E, L = 32, 128, 1024, 16, 8
NCORES = 8
OSL = H // NCORES            # 128 output columns of dense layer per core
NHC = H // 128               # 8 h-chunks per expert
NWC = E * NHC                # 128 weight chunks
EL = E * L                   # 128
CSC = 512.0                  # Z pre-scale; tanh applies 1/CSC

# xga fp16 [128, XGAW]: x chunks + pre-scaled gate broadcast (Z inputs)
XA_XT = 0                    # xt[p, hc*B+b] = x[b, hc*128+p]        (NHC*B)
XA_GS = XA_XT + NHC * B      # gs[p, e*B+b] = gates[b,e]*CSC         (E*B)
XGAW = XA_GS + E * B         # 768
# xgb fp16 [128, XGBW]: bias chunk + stage-2 tensors (landing last)
XB_ZT = 0                    # ztail[p, b] = gates[b,p]*CSC, p<E     (B)
XB_WB = XB_ZT + B            # wb[p, j] = dense_b[p, osl[j]], p<E    (OSL)
XB_OW = XB_WB + OSL          # ow[p, l*E+e] = out_proj_w[e,l,osl[p]] (EL)
XB_GX = XB_OW + EL           # gex[p, l*E+e] = gates[p,e], p<B       (EL)
XGBW = XB_GX + EL            # 416

# fp16 weight chunk groups, one HWDGE DMA each, last group small so the
# PE finishes right behind the stream.
W_GROUPS = [64, 32, 24, 8]
assert sum(W_GROUPS) == NWC

_CACHE = {}
LAST_RESULTS = None


def _build_nc():
    import concourse.bacc as bacc
    import concourse.tile as tile
    import concourse.mybir as mybir

    f16 = mybir.dt.float16
    f32 = mybir.dt.float32

    nc = bacc.Bacc("TRN2", target_bir_lowering=False, debug=False,
                   num_devices=NCORES)

    w_d = nc.dram_tensor("w", [128, NWC * OSL], f16, kind="ExternalInput")
    xga_d = nc.dram_tensor("xga", [128, XGAW], f16, kind="ExternalInput")
    xgb_d = nc.dram_tensor("xgb", [128, XGBW], f16, kind="ExternalInput")
    out_d = nc.dram_tensor("out", [B, L], f32, kind="ExternalOutput")

    with tile.TileContext(nc) as tc:
        with (
            tc.tile_pool(name="const", bufs=1) as cpool,
            tc.tile_pool(name="wz", bufs=1) as wpool,
            tc.tile_pool(name="work", bufs=1) as spool,
            tc.tile_pool(name="psum", bufs=1, space="PSUM") as ppool,
        ):
            # Input stream, one queue, FIFO: Z inputs first, then weight
            # groups big->small, then the tail tensors.
            xga_sb = cpool.tile([128, XGAW], f16)
            nc.sync.dma_start(xga_sb[:], xga_d[:])
            wts = []
            c0 = 0
            for g, n_c in enumerate(W_GROUPS):
                wt = wpool.tile([128, n_c * OSL], f16, tag=f"wt{g}",
                                name=f"wt{g}")
                nc.sync.dma_start(wt[:], w_d[:, c0 * OSL:(c0 + n_c) * OSL])
                wts.append((c0, n_c, wt))
                c0 += n_c
            xgb_sb = cpool.tile([128, XGBW], f16)
            nc.sync.dma_start(xgb_sb[:], xgb_d[:])

            # Preload the tanh table early on the Activation engine.
            dz = spool.tile([1, 1], f32)
            nc.vector.memset(dz[:], 0.0)
            dzo = spool.tile([1, 1], f16)
            nc.scalar.activation(dzo[:], dz[:],
                                 mybir.ActivationFunctionType.Tanh)

            # Z on DVE: one mul per expert, consumption-ordered.
            zt_sb = spool.tile([128, NWC * B], f16)
            xt3 = xga_sb[:, XA_XT:XA_XT + NHC * B].rearrange(
                "p (h b) -> p h b", b=B)
            for e in range(E):
                g_b = (
                    xga_sb[:, XA_GS + e * B:XA_GS + (e + 1) * B]
                    .unsqueeze(1)
                    .to_broadcast((128, NHC, B))
                )
                nc.vector.tensor_mul(
                    zt_sb[:, e * NHC * B:(e + 1) * NHC * B].rearrange(
                        "p (h b) -> p h b", b=B),
                    xt3,
                    g_b,
                )

            # Stage 1: h_preT[o, b] over 128 chunks + 1 bias chunk.
            ps1 = ppool.tile([OSL, B], f32)
            for c0, n_c, wt in wts:
                for i in range(n_c):
                    c = c0 + i
                    nc.tensor.matmul(
                        ps1[:],
                        wt[:, i * OSL:(i + 1) * OSL],
                        zt_sb[:, c * B:(c + 1) * B],
                        start=(c == 0),
                        stop=False,
                    )
            nc.tensor.matmul(
                ps1[:],
                xgb_sb[:, XB_WB:XB_WB + OSL],
                xgb_sb[:, XB_ZT:XB_ZT + B],
                start=False,
                stop=True,
            )

            ht = spool.tile([OSL, B], f16)
            nc.scalar.activation(ht[:], ps1[:],
                                 mybir.ActivationFunctionType.Tanh,
                                 scale=1.0 / CSC)

            ps2 = ppool.tile([B, EL], f32)
            nc.tensor.matmul(
                ps2[:], ht[:], xgb_sb[:, XB_OW:XB_OW + EL],
                start=True, stop=True)

            # r[b,(l,e)] = ps2 * gates[b,e]; reduce over e (innermost).
            r = spool.tile([B, EL], f32)
            nc.vector.tensor_mul(
                r[:], ps2[:], xgb_sb[0:B, XB_GX:XB_GX + EL])
            out_r = spool.tile([B, L], f32)
            nc.vector.tensor_reduce(
                out_r[:],
                r[:].rearrange("p (l e) -> p l e", e=E),
                axis=mybir.AxisListType.X,
                op=mybir.AluOpType.add,
            )
            nc.scalar.dma_start(out_d[:], out_r[:])

    nc.compile()
    return nc


def _get_nc():
    if "nc" not in _CACHE:
        _CACHE["nc"] = _build_nc()
    return _CACHE["nc"]


def make_in_maps(X, gates, dense_w, dense_b, out_proj_w, out_proj_b):
    """Host-side shard + pack. Returns (in_maps, host_bias)."""
    X = np.asarray(X, np.float32)
    gates = np.asarray(gates, np.float32)
    dense_w = np.asarray(dense_w, np.float32)
    dense_b = np.asarray(dense_b, np.float32)
    out_proj_w = np.asarray(out_proj_w, np.float32)
    out_proj_b = np.asarray(out_proj_b, np.float32)

    x = X[:, 0, :]                                     # [B, H]
    dw_t = dense_w.transpose(0, 2, 1)                  # [E, h, o]

    xga = np.zeros((128, XGAW), np.float16)
    # xt[p, hc*B+b] = x[b, hc*128+p]
    xga[:, XA_XT:XA_XT + NHC * B] = (
        x.T.reshape(NHC, 128, B).transpose(1, 0, 2).reshape(128, NHC * B)
    )
    xga[:, XA_GS:XA_GS + E * B] = np.broadcast_to(
        (gates * CSC).T.reshape(1, E * B), (128, E * B))

    xgb = np.zeros((128, XGBW), np.float16)
    xgb[:E, XB_ZT:XB_ZT + B] = gates.T * CSC           # bias-z rows
    xgb[:B, XB_GX:XB_GX + EL] = np.tile(gates, (1, L))

    in_maps = []
    for k in range(NCORES):
        sl = slice(k * OSL, (k + 1) * OSL)
        # w[p, c*OSL + j]: chunk c=(e,hc) holds dw_t[e, hc*128+p, sl][j]
        w = np.ascontiguousarray(
            dw_t[:, :, sl]
            .reshape(E, NHC, 128, OSL)
            .transpose(2, 0, 1, 3)
            .reshape(128, NWC * OSL)
        ).astype(np.float16)

        xgbk = xgb.copy()
        xgbk[:E, XB_WB:XB_WB + OSL] = dense_b[:, sl]
        xgbk[:, XB_OW:XB_OW + EL] = (
            out_proj_w[:, :, sl].transpose(2, 1, 0).reshape(OSL, EL)
        )

        in_maps.append({"w": w, "xga": xga, "xgb": xgbk})

    host_bias = (gates @ out_proj_b).astype(np.float32)   # [B, L]
    return in_maps, host_bias


def kernel(**inputs):
    global LAST_RESULTS
    from concourse.bass_utils import run_bass_kernel_spmd

    nc = _get_nc()
    in_maps, host_bias = make_in_maps(
        inputs["X"], inputs["gates"], inputs["dense_w"], inputs["dense_b"],
        inputs["out_proj_w"], inputs["out_proj_b"],
    )
    res = run_bass_kernel_spmd(nc, in_maps, list(range(NCORES)))
    LAST_RESULTS = res
    parts = [r["out"] for r in res.results]
    out = np.sum(parts, axis=0, dtype=np.float64).astype(np.float32) + host_bias
    return out
